# revision 1
# baseline (speedup 1.0000x reference)
"""CrossInvolution kernel for 8 Trainium2 NeuronCores.

Math (per batch b):
  t      = relu(bn(w1 @ guide))                       # [RED=64, H*W]
  weight = w2 @ t + b2                                # [G*K*K=784, H*W]
  out[c,p] = sum_k weight[g(c)*49+k, p] * x[c, p+dk] + x[c, p]

Sharding: 8 cores = 2 batches x 4 row-slices of 16 image rows each.
Each core computes its full pipeline on its slice (halo rows of the
feature map come in via host-side padding), so there is no duplicated
compute and no cross-core communication.

Device layout per core:
  - channels on partitions (2 halves of 128), pixels (16x64) on free dim
  - matmul2 output rows are reordered k-major (row m = k*16 + g) so the
    8 group-rows needed for one tap k sit on contiguous partitions;
    a constant one-hot matmul broadcasts them to all 128 channel lanes.
  - involution: 49 taps, each = PE broadcast (weights -> PSUM) then
    DVE multiply (shifted x window * broadcast weights) + accumulate.
"""

import numpy as np

import concourse.bass as bass
import concourse.bacc as bacc
import concourse.mybir as mybir
import concourse.tile as tile
from concourse.bass_utils import run_bass_kernel_spmd

FP = mybir.dt.float32
HP = mybir.dt.float16
N_CORES = 8
C = 256
RED = 64
G = 16
GC = 16
KS = 7
KK = KS * KS  # 49
H = W = 64
ROWS = 16          # image rows per core
PIX = ROWS * W     # 1024 pixels per core
PROWS = ROWS + 6   # padded rows (halo 3 each side)
PW = W + 6         # padded width

RSPLIT = 13        # rows on DVE; rest on GPSIMD

TRACE = False
LAST_RESULTS = None

_CACHED_NC = None


def _build_nc():
    nc = bacc.Bacc(
        "TRN2",
        debug=False,
        target_bir_lowering=False,
        num_devices=N_CORES,
    )

    guide = nc.dram_tensor("guide", (C, ROWS, W), HP, kind="ExternalInput")
    feat = nc.dram_tensor("feat", (C, PROWS, PW), FP, kind="ExternalInput")
    w1t = nc.dram_tensor("w1t", (C, RED), HP, kind="ExternalInput")
    w2r = nc.dram_tensor("w2r", (RED, 14 * 128), HP, kind="ExternalInput")
    b2r = nc.dram_tensor("b2r", (14 * 128,), FP, kind="ExternalInput")
    scl = nc.dram_tensor("scl", (RED, 1), FP, kind="ExternalInput")
    shf = nc.dram_tensor("shf", (RED, 1), FP, kind="ExternalInput")
    out = nc.dram_tensor("out", (C, ROWS, W), FP, kind="ExternalOutput")

    # Replicated one-hot broadcast matrices, one per channel half i:
    # rows 32t+g (t=0..3, g=0..15) hold B_i[g, c] = 1 iff c//16 + 8i == g,
    # so a tap slot at base partition 32t has a matching lhsT at the
    # same base partition.
    brep = []
    for i in range(2):
        b_np = np.zeros((128, 128), np.float16)
        for t in range(4):
            for g in range(GC):
                if 0 <= g - 8 * i < 8:
                    c0 = (g - 8 * i) * GC
                    b_np[32 * t + g, c0:c0 + GC] = 1.0
        brep.append(nc.inline_tensor(b_np, name=f"brep{i}"))

    NCHUNK = 14         # matmul2 chunks: j = 2*kw + b, taps kh = 4b+t
    MC = 128            # rows per chunk = up to 4 taps * 32-row-aligned slots

    with tile.TileContext(nc) as tc:
        with (
            tc.tile_pool(name="consts", bufs=1) as consts,
            tc.tile_pool(name="big", bufs=1) as big,
            tc.tile_pool(name="work", bufs=3) as work,
            tc.tile_pool(name="psum", bufs=2, space="PSUM") as psum,
        ):
            # ---- ACT table preload: tiny dep-free op at t=0 ----
            warm = consts.tile([1, 1], FP)
            nc.vector.memset(warm, 0.0)
            nc.scalar.activation(warm, warm,
                                 mybir.ActivationFunctionType.Relu)

            # ---- loads (order matters for pipeline head) ----
            scl_sb = consts.tile([RED, 1], FP)
            nc.sync.dma_start(scl_sb, scl.ap())
            shf_sb = consts.tile([RED, 1], FP)
            nc.sync.dma_start(shf_sb, shf.ap())
            w1t_sb = [consts.tile([128, RED], HP, tag=f"w1t{i}", name=f"w1t{i}") for i in range(2)]
            guide_sb = [[big.tile([128, ROWS // 2, W], HP, tag=f"gd{i}_{h}",
                                  name=f"gd{i}_{h}") for h in range(2)]
                        for i in range(2)]
            xp_sb = [big.tile([128, PROWS, PW], FP, tag=f"xp{i}", name=f"xp{i}") for i in range(2)]
            for h in range(2):
                for i in range(2):
                    csl = slice(i * 128, (i + 1) * 128)
                    nc.sync.dma_start(
                        guide_sb[i][h],
                        guide.ap()[csl, h * (ROWS // 2):(h + 1) * (ROWS // 2)])
            for i in range(2):
                nc.sync.dma_start(w1t_sb[i], w1t.ap()[i * 128:(i + 1) * 128])
            for i in range(2):
                csl = slice(i * 128, (i + 1) * 128)
                nc.sync.dma_start(xp_sb[i], feat.ap()[csl])
            w2r_sb = consts.tile([RED, 14 * 128], HP)
            nc.sync.dma_start(w2r_sb, w2r.ap())
            b_sb = [consts.tile([128, 128], HP, tag=f"bsb{i}", name=f"bsb{i}")
                    for i in range(2)]
            for i in range(2):
                nc.sync.dma_start(b_sb[i], brep[i].ap())
            b2_sb = consts.tile([MC, NCHUNK], FP)
            nc.sync.dma_start(b2_sb, b2r.ap().rearrange("(j p) -> p j", p=MC))


            # ---- matmul1 + BN/relu -> t (fp16) ----
            t_sb = big.tile([RED, PIX], HP)
            for nh in range(2):
                nsl = slice(nh * 512, (nh + 1) * 512)
                t_ps = psum.tile([RED, 512], FP, tag="wb", padded_shape=[RED, 2048])
                for i in range(2):
                    nc.tensor.matmul(
                        t_ps,
                        w1t_sb[i],
                        guide_sb[i][nh].rearrange("p a b -> p (a b)"),
                        start=(i == 0),
                        stop=(i == 1),
                    )
                nc.scalar.activation(
                    t_sb[:, nsl], t_ps,
                    mybir.ActivationFunctionType.Relu,
                    bias=shf_sb[:, :], scale=scl_sb[:, :],
                )

            # fp16 feature copies (even/odd alignment variants)
            xh_sb = [big.tile([128, PROWS * PW], HP, tag=f"xh{i}", name=f"xh{i}")
                     for i in range(2)]
            xo_sb = [big.tile([128, PROWS * PW], HP, tag=f"xo{i}", name=f"xo{i}")
                     for i in range(2)]
            xp_flat = [xp_sb[i].rearrange("p a b -> p (a b)") for i in range(2)]
            for i in range(2):
                nc.vector.tensor_copy(xh_sb[i], xp_flat[i])
                nc.vector.tensor_copy(
                    xo_sb[i][:, 0:PROWS * PW - 1], xp_flat[i][:, 1:PROWS * PW])

            # ---- involution ----
            # mm2 chunks: j = 2*kh + b; slot t: b=0 -> kw=2t (4 slots),
            # b=1 -> kw=2t+1 (3 slots). wbh slot s: s<4 -> kw=2s, else 2(s-4)+1.
            w_sb = [big.tile([MC, PIX], HP, tag=f"wsb{j}", name=f"wsb{j}")
                    for j in range(NCHUNK)]
            acc7 = [work.tile([128, KS * PIX], HP, tag=f"acc7_{i}",
                              name=f"acc7_{i}", bufs=1) for i in range(2)]
            GR = ROWS - RSPLIT
            acc7g = [work.tile([128, KS, GR, W], HP, tag=f"acc7g_{i}",
                               name=f"acc7g_{i}", bufs=1) for i in range(2)] \
                if GR > 0 else None
            acc = [work.tile([128, ROWS, W], FP, tag=f"acc{i}", name=f"acc{i}")
                   for i in range(2)]

            def emit_mm2(j):
                w_ps = psum.tile([MC, PIX], FP, tag="wb",
                                 padded_shape=[MC, 2048], name=f"wps{j}")
                for nh in range(2):
                    nsl = slice(nh * 512, (nh + 1) * 512)
                    nc.tensor.matmul(
                        w_ps[:, nsl],
                        w2r_sb[:, j * MC:(j + 1) * MC],
                        t_sb[:, nsl],
                        start=True,
                        stop=True,
                    )
                nc.scalar.activation(
                    w_sb[j], w_ps,
                    mybir.ActivationFunctionType.Identity,
                    bias=b2_sb[:, j:j + 1], scale=1.0,
                )

            SLOT_PAIRS = [(0, 2), (2, 2), (4, 2), (6, 1)]
            for i in range(2):
                for kh in range(KS):
                    if i == 0:
                        emit_mm2(2 * kh)
                        emit_mm2(2 * kh + 1)
                    wbhE = work.tile([128, 4 * PIX], HP, tag="wbhE", bufs=2)
                    wbhO = work.tile([128, 3 * PIX], HP, tag="wbhO", bufs=2)
                    for (s0, ns) in SLOT_PAIRS:
                        wb = psum.tile([128, ns * PIX], FP, tag="wb",
                                       padded_shape=[128, 2 * PIX])
                        for u in range(ns):
                            sl = s0 + u
                            j = 2 * kh + (0 if sl < 4 else 1)
                            t = sl if sl < 4 else sl - 4
                            for nh in range(2):
                                nsl = slice(u * PIX + nh * 512,
                                            u * PIX + nh * 512 + 512)
                                nc.tensor.matmul(
                                    wb[:, nsl],
                                    b_sb[i][32 * t:32 * t + G, :],
                                    w_sb[j][32 * t:32 * t + G,
                                            nh * 512:nh * 512 + 512],
                                    start=True,
                                    stop=True,
                                    tile_position=(32 * t, 0),
                                )
                        dst = (wbhE[:, s0 * PIX:(s0 + ns) * PIX] if s0 < 4
                               else wbhO[:, (s0 - 4) * PIX:(s0 - 4 + ns) * PIX])
                        if ns == 1 and kh in (2, 3, 4, 5):
                            nc.vector.tensor_copy(dst, wb)
                        else:
                            nc.scalar.activation(
                                dst, wb,
                                mybir.ActivationFunctionType.Copy,
                            )
                    # slots 0..3 = even kw (from xh), 4..6 = odd (xo);
                    # DVE takes rows [0, RSPLIT), GPSIMD rows [RSPLIT, ROWS)
                    o = kh * PW
                    for (s0, cnt, src, o2) in (
                        (0, 4, xh_sb[i], o),
                        (4, 3, xo_sb[i], o),
                    ):
                        wtile = wbhE if s0 < 4 else wbhO
                        ws0 = s0 if s0 < 4 else 0
                        wbh4 = wtile.rearrange("p (a b c) -> p a b c",
                                               b=ROWS, c=W)
                        prod = prodg = None
                        if kh != 0:
                            prod = work.tile([128, cnt * PIX], HP, tag="prod",
                                             padded_shape=[128, 4 * PIX])
                            if GR > 0:
                                prodg = work.tile([128, cnt, GR, W], HP,
                                                  tag="prodg",
                                                  padded_shape=[128, 4, GR, W])
                        for (eng, r0, rn) in (
                            (nc.vector, 0, RSPLIT),
                            (nc.gpsimd, RSPLIT, ROWS - RSPLIT),
                        ):
                            if rn <= 0:
                                continue
                            xs = bass.AP(
                                tensor=src.tensor,
                                offset=src.offset + o2 + r0 * PW,
                                ap=[src.ap[0], [2, cnt], [PW, rn], [1, W]],
                            )
                            wslc = wbh4[:, ws0:ws0 + cnt, r0:r0 + rn, :]
                            if eng is nc.vector:
                                aslc = acc7[i].rearrange(
                                    "p (a b c) -> p a b c", b=ROWS, c=W)[
                                    :, s0:s0 + cnt, r0:r0 + rn, :]
                                pslc = None if prod is None else prod.rearrange(
                                    "p (a b c) -> p a b c", b=ROWS, c=W)[
                                    :, 0:cnt, r0:r0 + rn, :]
                            else:
                                aslc = acc7g[i][:, s0:s0 + cnt, :, :]
                                pslc = None if prodg is None else \
                                    prodg[:, 0:cnt, :, :]
                            if kh == 0:
                                eng.tensor_tensor(
                                    aslc, xs, wslc, mybir.AluOpType.mult)
                            else:
                                eng.tensor_tensor(
                                    pslc, xs, wslc, mybir.AluOpType.mult)
                                eng.tensor_tensor(
                                    aslc, aslc, pslc, mybir.AluOpType.add)
                # ---- per-half tail: lane tree + residual ----
                a4 = acc7[i].rearrange("p (a b c) -> p a b c", b=ROWS, c=W)
                s03 = work.tile([128, 2, RSPLIT, W], HP, tag="s03")
                nc.vector.tensor_tensor(
                    s03, a4[:, 0:2, 0:RSPLIT], a4[:, 2:4, 0:RSPLIT],
                    mybir.AluOpType.add)
                s45 = work.tile([128, RSPLIT, W], HP, tag="s45")
                nc.vector.tensor_tensor(
                    s45, a4[:, 4, 0:RSPLIT], a4[:, 5, 0:RSPLIT],
                    mybir.AluOpType.add)
                nc.vector.tensor_tensor(
                    s45, s45, a4[:, 6, 0:RSPLIT], mybir.AluOpType.add)
                nc.vector.tensor_tensor(
                    s45, s45, s03[:, 0], mybir.AluOpType.add)
                nc.vector.tensor_tensor(
                    s45, s45, s03[:, 1], mybir.AluOpType.add)
                nc.vector.tensor_copy(acc[i], xp_sb[i][:, 3:3 + ROWS, 3:3 + W])
                nc.vector.tensor_tensor(
                    acc[i][:, 0:RSPLIT, :], acc[i][:, 0:RSPLIT, :], s45,
                    mybir.AluOpType.add)
                if GR > 0:
                    sg = work.tile([128, 2, GR, W], HP, tag="sg")
                    nc.gpsimd.tensor_tensor(
                        sg, acc7g[i][:, 0:2], acc7g[i][:, 2:4],
                        mybir.AluOpType.add)
                    sg2 = work.tile([128, GR, W], HP, tag="sg2")
                    nc.gpsimd.tensor_tensor(
                        sg2, acc7g[i][:, 4], acc7g[i][:, 5],
                        mybir.AluOpType.add)
                    nc.gpsimd.tensor_tensor(
                        sg2, sg2, acc7g[i][:, 6], mybir.AluOpType.add)
                    nc.gpsimd.tensor_tensor(
                        sg2, sg2, sg[:, 0], mybir.AluOpType.add)
                    nc.gpsimd.tensor_tensor(
                        sg2, sg2, sg[:, 1], mybir.AluOpType.add)
                    nc.vector.tensor_tensor(
                        acc[i][:, RSPLIT:ROWS, :], acc[i][:, RSPLIT:ROWS, :],
                        sg2, mybir.AluOpType.add)
                nc.sync.dma_start(out.ap()[i * 128:(i + 1) * 128], acc[i])

    nc.compile()
    return nc


def kernel(**inputs):
    global _CACHED_NC, LAST_RESULTS
    feature_map = np.asarray(inputs["feature_map"], np.float32)
    guide_map = np.asarray(inputs["guide_map"], np.float32)
    w1 = np.asarray(inputs["w1"], np.float32)
    bn_gamma = np.asarray(inputs["bn_gamma"], np.float32)
    bn_beta = np.asarray(inputs["bn_beta"], np.float32)
    bn_mean = np.asarray(inputs["bn_mean"], np.float32)
    bn_var = np.asarray(inputs["bn_var"], np.float32)
    w2 = np.asarray(inputs["w2"], np.float32)
    b2 = np.asarray(inputs["b2"], np.float32)

    scale = bn_gamma / np.sqrt(bn_var + 1e-5)
    shift = bn_beta - bn_mean * scale
    w1t = np.ascontiguousarray(w1.T).astype(np.float16)    # [256, 64]
    # lhsT for matmul2: col m = 128*j + 32*t + g for tap k = 4*j + t
    # (slots g=16..31 of each 32-row block are zero padding)
    w2gk = w2.reshape(G, KK, RED)        # [g, k, r]
    w2r = np.zeros((RED, 14, 4, 32), np.float32)
    b2r = np.zeros((14, 4, 32), np.float32)
    for k in range(KK):
        kh, kw = divmod(k, KS)
        j = 2 * kh + (kw % 2)
        t = kw // 2
        w2r[:, j, t, :G] = w2gk[:, k, :].T
        b2r[j, t, :G] = b2.reshape(G, KK)[:, k]
    w2r = np.ascontiguousarray(w2r.reshape(RED, 14 * 128)).astype(np.float16)
    b2r = np.ascontiguousarray(b2r.reshape(14 * 128))

    fpad = np.pad(feature_map, ((0, 0), (0, 0), (3, 3), (3, 3)))

    in_maps = []
    for core in range(N_CORES):
        b, q = divmod(core, 4)
        r0 = q * ROWS
        in_maps.append(dict(
            guide=np.ascontiguousarray(
                guide_map[b, :, r0:r0 + ROWS, :]).astype(np.float16),
            feat=np.ascontiguousarray(fpad[b, :, r0:r0 + PROWS, :]),
            w1t=w1t, w2r=w2r, b2r=b2r,
            scl=scale.reshape(-1, 1), shf=shift.reshape(-1, 1),
        ))

    if _CACHED_NC is None:
        _CACHED_NC = _build_nc()
    nc = _CACHED_NC

    br = run_bass_kernel_spmd(
        nc, in_maps, list(range(N_CORES)), trace=TRACE,
    )
    LAST_RESULTS = br

    out = np.empty((2, C, H, W), np.float32)
    for core in range(N_CORES):
        b, q = divmod(core, 4)
        r0 = q * ROWS
        out[b, :, r0:r0 + ROWS, :] = br.results[core]["out"]
    return out



# revision 2
# speedup vs baseline: 1.3150x; 1.3150x over previous
"""CrossInvolution kernel for 8 Trainium2 NeuronCores.

Math (per batch b):
  t      = relu(bn(w1 @ guide))                       # [RED=64, H*W]
  weight = w2 @ t + b2                                # [G*K*K=784, H*W]
  out[c,p] = sum_k weight[g(c)*49+k, p] * x[c, p+dk] + x[c, p]

Sharding: 8 cores = 2 batches x 4 row-slices of 16 image rows each.
Each core computes its full pipeline on its slice (halo rows come in
via host-side padding); no cross-core communication.

Engine plan (v2):
  - PE: mm1, then one fused "broadcast" matmul per tap with
    lhsT = w2e[65,128] (w2 rearranged per channel; row 64 carries the
    b2 bias via a ones-row appended to t; center tap bias += 1 folds
    in the +x residual). PSUM directly holds per-channel tap weights.
  - Scalar (ACT): BN+relu on t, PSUM->SBUF fp16 weight copies.
  - DVE: the involution multiplies and adds only, all contiguous fp16
    SBUF ops (2x mode). x arrives as 7 kw-shifted contiguous copies
    DMA'd straight from HBM (host ships the padded fp16 feature map).
  - GPSIMD: unused (concurrent GPSIMD SBUF reads starve the DVE).
"""

import numpy as np

import concourse.bass as bass
import concourse.bacc as bacc
import concourse.mybir as mybir
import concourse.tile as tile
from concourse.bass_utils import run_bass_kernel_spmd

FP = mybir.dt.float32
HP = mybir.dt.float16
N_CORES = 8
C = 256
RED = 64
G = 16
GC = 16
KS = 7
KK = KS * KS  # 49
H = W = 64
ROWS = 16          # image rows per core
PIX = ROWS * W     # 1024 pixels per core
PROWS = ROWS + 6   # padded rows (halo 3 each side)
PW = W + 6         # padded width
XROW = PROWS * W   # 1408: one kw-shifted copy, rows contiguous at 64

TRACE = False
LAST_RESULTS = None

_CACHED_NC = None


def _build_nc():
    nc = bacc.Bacc(
        "TRN2",
        debug=False,
        target_bir_lowering=False,
        num_devices=N_CORES,
    )

    guide = nc.dram_tensor("guide", (C, ROWS, W), HP, kind="ExternalInput")
    feat = nc.dram_tensor("feat", (C, PROWS, PW), HP, kind="ExternalInput")
    w1t = nc.dram_tensor("w1t", (C, RED), HP, kind="ExternalInput")
    w2a = nc.dram_tensor("w2a", (RED + 1, 2 * KK * 128), HP,
                         kind="ExternalInput")
    scl = nc.dram_tensor("scl", (RED, 1), FP, kind="ExternalInput")
    shf = nc.dram_tensor("shf", (RED, 1), FP, kind="ExternalInput")
    out = nc.dram_tensor("out", (C, ROWS, W), HP, kind="ExternalOutput")

    MUL = mybir.AluOpType.mult
    ADD = mybir.AluOpType.add
    # taps per PSUM chunk: {2,2,2,1} per kw-group (PSUM = 8 banks =
    # 4096 fp32; 2-tap chunk = 2048 fp32 = 4 banks, ping-ponged)
    CHUNKS = [(0, 2), (2, 2), (4, 2), (6, 1)]

    with tile.TileContext(nc) as tc:
        with (
            tc.tile_pool(name="consts", bufs=1) as consts,
            tc.tile_pool(name="big", bufs=1) as big,
            tc.tile_pool(name="work", bufs=3) as work,
            tc.tile_pool(name="psum", bufs=2, space="PSUM") as psum,
        ):
            # ---- ACT table preload: tiny dep-free op at t=0 ----
            warm = consts.tile([1, 1], FP)
            nc.vector.memset(warm, 0.0)
            nc.scalar.activation(warm, warm,
                                 mybir.ActivationFunctionType.Relu)

            # ---- loads ----
            scl_sb = consts.tile([RED, 1], FP)
            nc.sync.dma_start(scl_sb, scl.ap())
            shf_sb = consts.tile([RED, 1], FP)
            nc.sync.dma_start(shf_sb, shf.ap())
            w1t_sb = [consts.tile([128, RED], HP, tag=f"w1t{i}",
                                  name=f"w1t{i}") for i in range(2)]
            guide_sb = [big.tile([128, PIX], HP, tag=f"gd{i}", name=f"gd{i}")
                        for i in range(2)]
            for i in range(2):
                csl = slice(i * 128, (i + 1) * 128)
                nc.sync.dma_start(
                    guide_sb[i],
                    guide.ap()[csl].rearrange("p a b -> p (a b)"))
                nc.sync.dma_start(w1t_sb[i], w1t.ap()[i * 128:(i + 1) * 128])
            w2a_sb = consts.tile([RED + 1, 2 * KK * 128], HP)
            nc.sync.dma_start(w2a_sb, w2a.ap())
            # 7 kw-shifted fp16 feature copies per half, rows contiguous
            # at width 64: xk[i][:, kw*XROW + r*64 + x] = feat[c, r, x+kw]
            xk = [big.tile([128, KS * XROW], HP, tag=f"xk{i}", name=f"xk{i}")
                  for i in range(2)]
            for i in range(2):
                csl = slice(i * 128, (i + 1) * 128)
                for kw in range(KS):
                    nc.sync.dma_start(
                        xk[i][:, kw * XROW:(kw + 1) * XROW],
                        feat.ap()[csl, :, kw:kw + W])

            # ---- mm1 + BN/relu -> t (fp16), plus ones row for bias ----
            t_sb = big.tile([RED + 1, PIX], HP)
            nc.vector.memset(t_sb[RED:RED + 1, :], 1.0)
            for nh in range(2):
                nsl = slice(nh * 512, (nh + 1) * 512)
                t_ps = psum.tile([RED, 512], FP, tag="wb",
                                 padded_shape=[RED, 2048])
                for i in range(2):
                    nc.tensor.matmul(
                        t_ps,
                        w1t_sb[i],
                        guide_sb[i][:, nsl],
                        start=(i == 0),
                        stop=(i == 1),
                    )
                nc.scalar.activation(
                    t_sb[0:RED, nsl], t_ps,
                    mybir.ActivationFunctionType.Relu,
                    bias=shf_sb[:, :], scale=scl_sb[:, :],
                )

            # ---- involution ----
            for i in range(2):
                acc7 = big.tile([128, KS * PIX], HP, tag=f"acc7_{i}",
                                name=f"acc7_{i}")
                for kw in range(KS):
                    # broadcast matmuls + fp16 copy for the 7 kh-taps
                    wsb = work.tile([128, KS * PIX], HP, tag="wsb", bufs=2)
                    for (k0, nk) in CHUNKS:
                        w_ps = psum.tile([128, nk * PIX], FP, tag="wb",
                                         padded_shape=[128, 2048])
                        for u in range(nk):
                            k = (k0 + u) * KS + kw   # tap = kh*7+kw
                            lhs = w2a_sb[:, (i * KK + k) * 128:
                                         (i * KK + k + 1) * 128]
                            for nh in range(2):
                                nc.tensor.matmul(
                                    w_ps[:, u * PIX + nh * 512:
                                         u * PIX + nh * 512 + 512],
                                    lhs,
                                    t_sb[:, nh * 512:nh * 512 + 512],
                                    start=True,
                                    stop=True,
                                )
                        nc.scalar.activation(
                            wsb[:, k0 * PIX:(k0 + nk) * PIX], w_ps,
                            mybir.ActivationFunctionType.Copy,
                        )
                    # multiply all 7 kh-taps: runs of 1024 contiguous
                    xs = bass.AP(
                        tensor=xk[i].tensor,
                        offset=xk[i].offset + kw * XROW,
                        ap=[xk[i].ap[0], [W, KS], [1, PIX]],
                    )
                    prod = work.tile([128, KS * PIX], HP, tag="prod", bufs=2)
                    nc.vector.tensor_tensor(
                        prod.rearrange("p (a b) -> p a b", b=PIX),
                        xs,
                        wsb.rearrange("p (a b) -> p a b", b=PIX),
                        MUL)
                    # 7 -> 1 tree, all contiguous fp16
                    s2 = work.tile([128, 3 * PIX], HP, tag="s2", bufs=2)
                    nc.vector.tensor_tensor(
                        s2, prod[:, 0:3 * PIX], prod[:, 3 * PIX:6 * PIX], ADD)
                    ab = work.tile([128, 2 * PIX], HP, tag="ab", bufs=2)
                    nc.vector.tensor_tensor(
                        ab[:, 0:PIX], s2[:, 0:PIX], s2[:, PIX:2 * PIX], ADD)
                    nc.vector.tensor_tensor(
                        ab[:, PIX:2 * PIX], s2[:, 2 * PIX:3 * PIX],
                        prod[:, 6 * PIX:7 * PIX], ADD)
                    nc.vector.tensor_tensor(
                        acc7[:, kw * PIX:(kw + 1) * PIX],
                        ab[:, 0:PIX], ab[:, PIX:2 * PIX], ADD)
                # cross-kw tree (7 -> 1) + store
                u3 = work.tile([128, 3 * PIX], HP, tag="s2", bufs=2)
                nc.vector.tensor_tensor(
                    u3, acc7[:, 0:3 * PIX], acc7[:, 3 * PIX:6 * PIX], ADD)
                uv = work.tile([128, 2 * PIX], HP, tag="ab", bufs=2)
                nc.vector.tensor_tensor(
                    uv[:, 0:PIX], u3[:, 0:PIX], u3[:, PIX:2 * PIX], ADD)
                nc.vector.tensor_tensor(
                    uv[:, PIX:2 * PIX], u3[:, 2 * PIX:3 * PIX],
                    acc7[:, 6 * PIX:7 * PIX], ADD)
                osb = big.tile([128, PIX], HP, tag=f"osb{i}", name=f"osb{i}")
                nc.vector.tensor_tensor(
                    osb, uv[:, 0:PIX], uv[:, PIX:2 * PIX], ADD)
                nc.sync.dma_start(
                    out.ap()[i * 128:(i + 1) * 128].rearrange(
                        "p a b -> p (a b)"),
                    osb)

    nc.compile()
    return nc


def kernel(**inputs):
    global _CACHED_NC, LAST_RESULTS
    feature_map = np.asarray(inputs["feature_map"], np.float32)
    guide_map = np.asarray(inputs["guide_map"], np.float32)
    w1 = np.asarray(inputs["w1"], np.float32)
    bn_gamma = np.asarray(inputs["bn_gamma"], np.float32)
    bn_beta = np.asarray(inputs["bn_beta"], np.float32)
    bn_mean = np.asarray(inputs["bn_mean"], np.float32)
    bn_var = np.asarray(inputs["bn_var"], np.float32)
    w2 = np.asarray(inputs["w2"], np.float32)
    b2 = np.asarray(inputs["b2"], np.float32)

    scale = bn_gamma / np.sqrt(bn_var + 1e-5)
    shift = bn_beta - bn_mean * scale
    w1t = np.ascontiguousarray(w1.T).astype(np.float16)    # [256, 64]

    # w2a[r, i, k, c] = w2[(c//16 + 8i)*49 + k, r]; row RED = b2 bias
    # (+1 on the center tap: folds the +x residual into the involution)
    w2g = w2.reshape(G, KK, RED)                            # [g, k, r]
    b2g = b2.reshape(G, KK).copy()                          # [g, k]
    b2g[:, (KK - 1) // 2] += 1.0
    w2a = np.zeros((RED + 1, 2, KK, 128), np.float32)
    for i in range(2):
        gidx = np.arange(128) // GC + 8 * i                 # [c] -> g
        w2a[0:RED, i] = w2g[gidx].transpose(2, 1, 0)        # [r, k, c]
        w2a[RED, i] = b2g[gidx].T                           # [k, c]
    w2a = np.ascontiguousarray(
        w2a.reshape(RED + 1, 2 * KK * 128)).astype(np.float16)

    fpad = np.pad(feature_map, ((0, 0), (0, 0), (3, 3), (3, 3))).astype(
        np.float16)

    in_maps = []
    for core in range(N_CORES):
        b, q = divmod(core, 4)
        r0 = q * ROWS
        in_maps.append(dict(
            guide=np.ascontiguousarray(
                guide_map[b, :, r0:r0 + ROWS, :]).astype(np.float16),
            feat=np.ascontiguousarray(fpad[b, :, r0:r0 + PROWS, :]),
            w1t=w1t, w2a=w2a,
            scl=scale.reshape(-1, 1), shf=shift.reshape(-1, 1),
        ))

    if _CACHED_NC is None:
        _CACHED_NC = _build_nc()
    nc = _CACHED_NC

    br = run_bass_kernel_spmd(
        nc, in_maps, list(range(N_CORES)), trace=TRACE,
    )
    LAST_RESULTS = br

    out = np.empty((2, C, H, W), np.float32)
    for core in range(N_CORES):
        b, q = divmod(core, 4)
        r0 = q * ROWS
        out[b, :, r0:r0 + ROWS, :] = br.results[core]["out"].astype(
            np.float32)
    return out


# revision 7
# speedup vs baseline: 1.3826x; 1.0514x over previous
"""CrossInvolution kernel for 8 Trainium2 NeuronCores.

Math (per batch b):
  t      = relu(bn(w1 @ guide))                       # [RED=64, H*W]
  weight = w2 @ t + b2                                # [G*K*K=784, H*W]
  out[c,p] = sum_k weight[g(c)*49+k, p] * x[c, p+dk] + x[c, p]

Sharding: 8 cores = 2 batches x 4 row-slices of 16 image rows each.
Each core computes its full pipeline on its slice (halo rows come in
via host-side padding); no cross-core communication.

Engine plan (v2):
  - PE: mm1, then one fused "broadcast" matmul per tap with
    lhsT = w2e[65,128] (w2 rearranged per channel; row 64 carries the
    b2 bias via a ones-row appended to t; center tap bias += 1 folds
    in the +x residual). PSUM directly holds per-channel tap weights.
  - Scalar (ACT): BN+relu on t, PSUM->SBUF fp16 weight copies.
  - DVE: the involution multiplies and adds only, all contiguous fp16
    SBUF ops (2x mode). x arrives as 7 kw-shifted contiguous copies
    DMA'd straight from HBM (host ships the padded fp16 feature map).
  - GPSIMD: unused (concurrent GPSIMD SBUF reads starve the DVE).
"""

import numpy as np

import concourse.bass as bass
import concourse.bacc as bacc
import concourse.mybir as mybir
import concourse.tile as tile
from concourse.bass_utils import run_bass_kernel_spmd

FP = mybir.dt.float32
HP = mybir.dt.float16
N_CORES = 8
C = 256
RED = 64
G = 16
GC = 16
KS = 7
KK = KS * KS  # 49
H = W = 64
ROWS = 16          # image rows per core
PIX = ROWS * W     # 1024 pixels per core
PROWS = ROWS + 6   # padded rows (halo 3 each side)
PW = W + 6         # padded width
XROW = PROWS * W   # 1408: one kw-shifted copy, rows contiguous at 64

TRACE = False
LAST_RESULTS = None

_CACHED_NC = None


def _build_nc():
    nc = bacc.Bacc(
        "TRN2",
        debug=False,
        target_bir_lowering=False,
        num_devices=N_CORES,
    )

    guide = nc.dram_tensor("guide", (C, ROWS, W), HP, kind="ExternalInput")
    # feat7[c, kw, r, x] = padded_feature[c, r, x + kw] (host-shifted, so
    # every load and every DVE multiply is contiguous at 64-wide rows)
    feat7 = nc.dram_tensor("feat7", (C, KS, PROWS, W), HP,
                           kind="ExternalInput")
    w1t = nc.dram_tensor("w1t", (C, RED), HP, kind="ExternalInput")
    # w2a[r, ((i*7+kw)*7+kh)*128 + c] = w2[(c//16+8i)*49 + kh*7+kw, r];
    # row RED carries b2 (+1 on center tap -> +x residual)
    w2a = nc.dram_tensor("w2a", (RED + 1, 2 * KK * 128), HP,
                         kind="ExternalInput")
    scl = nc.dram_tensor("scl", (RED, 1), FP, kind="ExternalInput")
    shf = nc.dram_tensor("shf", (RED, 1), FP, kind="ExternalInput")
    out = nc.dram_tensor("out", (C, ROWS, W), HP, kind="ExternalOutput")

    MUL = mybir.AluOpType.mult
    ADD = mybir.AluOpType.add
    # taps per PSUM chunk: {2,2,2,1} per kw-group (PSUM = 8 banks =
    # 4096 fp32; 2-tap chunk = 2048 fp32 = 4 banks, ping-ponged)
    CHUNKS = [(0, 2), (2, 2), (4, 2), (6, 1)]

    with tile.TileContext(nc) as tc:
        with (
            tc.tile_pool(name="consts", bufs=1) as consts,
            tc.tile_pool(name="big", bufs=1) as big,
            tc.tile_pool(name="work", bufs=3) as work,
            tc.tile_pool(name="psum", bufs=2, space="PSUM") as psum,
        ):
            # ---- ACT table preload: tiny dep-free op at t=0 ----
            warm = consts.tile([1, 1], FP)
            nc.vector.memset(warm, 0.0)
            nc.scalar.activation(warm, warm,
                                 mybir.ActivationFunctionType.Relu)

            # ---- loads ----
            scl_sb = consts.tile([RED, 1], FP)
            nc.sync.dma_start(scl_sb, scl.ap())
            shf_sb = consts.tile([RED, 1], FP)
            nc.sync.dma_start(shf_sb, shf.ap())
            w1t_sb = [consts.tile([128, RED], HP, tag=f"w1t{i}",
                                  name=f"w1t{i}") for i in range(2)]
            guide_sb = [big.tile([128, PIX], HP, tag=f"gd{i}", name=f"gd{i}")
                        for i in range(2)]
            for i in range(2):
                csl = slice(i * 128, (i + 1) * 128)
                nc.sync.dma_start(
                    guide_sb[i],
                    guide.ap()[csl].rearrange("p a b -> p (a b)"))
                nc.sync.dma_start(w1t_sb[i], w1t.ap()[i * 128:(i + 1) * 128])
            w2a_sb = consts.tile([RED + 1, 2 * KK * 128], HP)
            # per-(half, kw) chunks of w2a and xk stream in just-in-time
            # (emitted inside the kw loop, after the first groups' chunks)
            xk = [big.tile([128, KS * XROW], HP, tag=f"xk{i}", name=f"xk{i}")
                  for i in range(2)]

            def load_group(i, kw):
                csl = slice(i * 128, (i + 1) * 128)
                nc.sync.dma_start(
                    w2a_sb[:, (i * KS + kw) * KS * 128:
                           (i * KS + kw + 1) * KS * 128],
                    w2a.ap()[:, (i * KS + kw) * KS * 128:
                             (i * KS + kw + 1) * KS * 128])
                nc.sync.dma_start(
                    xk[i][:, kw * XROW:(kw + 1) * XROW],
                    feat7.ap()[csl, kw].rearrange("p a b -> p (a b)"))

            # first two groups' weight/feature chunks go on the queue now
            load_group(0, 0)
            load_group(0, 1)

            # ---- mm1 + BN/relu -> t (fp16), plus ones row for bias ----
            t_sb = big.tile([RED + 1, PIX], HP)
            nc.vector.memset(t_sb[RED:RED + 1, :], 1.0)
            for nh in range(2):
                nsl = slice(nh * 512, (nh + 1) * 512)
                t_ps = psum.tile([RED, 512], FP, tag="wb",
                                 padded_shape=[RED, 2048])
                for i in range(2):
                    nc.tensor.matmul(
                        t_ps,
                        w1t_sb[i],
                        guide_sb[i][:, nsl],
                        start=(i == 0),
                        stop=(i == 1),
                    )
                nc.scalar.activation(
                    t_sb[0:RED, nsl], t_ps,
                    mybir.ActivationFunctionType.Relu,
                    bias=shf_sb[:, :], scale=scl_sb[:, :],
                )

            # ---- involution ----
            for i in range(2):
                acc7 = big.tile([128, KS * PIX], HP, tag=f"acc7_{i}",
                                name=f"acc7_{i}")
                for kw in range(KS):
                    # prefetch the group two steps ahead
                    nxt = i * KS + kw + 2
                    if nxt < 2 * KS:
                        load_group(nxt // KS, nxt % KS)
                    # broadcast matmuls + fp16 copy for the 7 kh-taps
                    wsb = work.tile([128, KS * PIX], HP, tag="wsb", bufs=2)
                    for (k0, nk) in CHUNKS:
                        w_ps = psum.tile([128, nk * PIX], FP, tag="wb",
                                         padded_shape=[128, 2048])
                        for u in range(nk):
                            kh = k0 + u
                            lhs = w2a_sb[:, ((i * KS + kw) * KS + kh) * 128:
                                         ((i * KS + kw) * KS + kh + 1) * 128]
                            for nh in range(2):
                                nc.tensor.matmul(
                                    w_ps[:, u * PIX + nh * 512:
                                         u * PIX + nh * 512 + 512],
                                    lhs,
                                    t_sb[:, nh * 512:nh * 512 + 512],
                                    start=True,
                                    stop=True,
                                )
                        nc.scalar.activation(
                            wsb[:, k0 * PIX:(k0 + nk) * PIX], w_ps,
                            mybir.ActivationFunctionType.Copy,
                        )
                    # multiply all 7 kh-taps: runs of 1024 contiguous
                    xs = bass.AP(
                        tensor=xk[i].tensor,
                        offset=xk[i].offset + kw * XROW,
                        ap=[xk[i].ap[0], [W, KS], [1, PIX]],
                    )
                    prod = work.tile([128, KS * PIX], HP, tag="prod", bufs=2)
                    nc.vector.tensor_tensor(
                        prod.rearrange("p (a b) -> p a b", b=PIX),
                        xs,
                        wsb.rearrange("p (a b) -> p a b", b=PIX),
                        MUL)
                    # 7 -> 1 tree, all contiguous fp16
                    s2 = work.tile([128, 3 * PIX], HP, tag="s2", bufs=2)
                    nc.vector.tensor_tensor(
                        s2, prod[:, 0:3 * PIX], prod[:, 3 * PIX:6 * PIX], ADD)
                    ab = work.tile([128, 2 * PIX], HP, tag="ab", bufs=2)
                    nc.vector.tensor_tensor(
                        ab[:, 0:PIX], s2[:, 0:PIX], s2[:, PIX:2 * PIX], ADD)
                    nc.vector.tensor_tensor(
                        ab[:, PIX:2 * PIX], s2[:, 2 * PIX:3 * PIX],
                        prod[:, 6 * PIX:7 * PIX], ADD)
                    nc.vector.tensor_tensor(
                        acc7[:, kw * PIX:(kw + 1) * PIX],
                        ab[:, 0:PIX], ab[:, PIX:2 * PIX], ADD)
                # cross-kw tree (7 -> 1) + store
                u3 = work.tile([128, 3 * PIX], HP, tag="s2", bufs=2)
                nc.vector.tensor_tensor(
                    u3, acc7[:, 0:3 * PIX], acc7[:, 3 * PIX:6 * PIX], ADD)
                uv = work.tile([128, 2 * PIX], HP, tag="ab", bufs=2)
                nc.vector.tensor_tensor(
                    uv[:, 0:PIX], u3[:, 0:PIX], u3[:, PIX:2 * PIX], ADD)
                nc.vector.tensor_tensor(
                    uv[:, PIX:2 * PIX], u3[:, 2 * PIX:3 * PIX],
                    acc7[:, 6 * PIX:7 * PIX], ADD)
                osb = big.tile([128, PIX], HP, tag=f"osb{i}", name=f"osb{i}")
                nc.vector.tensor_tensor(
                    osb, uv[:, 0:PIX], uv[:, PIX:2 * PIX], ADD)
                nc.sync.dma_start(
                    out.ap()[i * 128:(i + 1) * 128].rearrange(
                        "p a b -> p (a b)"),
                    osb)

    nc.compile()
    return nc


def kernel(**inputs):
    global _CACHED_NC, LAST_RESULTS
    feature_map = np.asarray(inputs["feature_map"], np.float32)
    guide_map = np.asarray(inputs["guide_map"], np.float32)
    w1 = np.asarray(inputs["w1"], np.float32)
    bn_gamma = np.asarray(inputs["bn_gamma"], np.float32)
    bn_beta = np.asarray(inputs["bn_beta"], np.float32)
    bn_mean = np.asarray(inputs["bn_mean"], np.float32)
    bn_var = np.asarray(inputs["bn_var"], np.float32)
    w2 = np.asarray(inputs["w2"], np.float32)
    b2 = np.asarray(inputs["b2"], np.float32)

    scale = bn_gamma / np.sqrt(bn_var + 1e-5)
    shift = bn_beta - bn_mean * scale
    w1t = np.ascontiguousarray(w1.T).astype(np.float16)    # [256, 64]

    # w2a[r, i, kw, kh, c] = w2[(c//16 + 8i)*49 + kh*7+kw, r]; row RED =
    # b2 bias (+1 on the center tap: folds the +x residual in)
    w2g = w2.reshape(G, KS, KS, RED)                        # [g, kh, kw, r]
    b2g = b2.reshape(G, KS, KS).copy()                      # [g, kh, kw]
    b2g[:, 3, 3] += 1.0
    w2a = np.zeros((RED + 1, 2, KS, KS, 128), np.float32)
    for i in range(2):
        gidx = np.arange(128) // GC + 8 * i                 # [c] -> g
        # [r, kw, kh, c]
        w2a[0:RED, i] = w2g[gidx].transpose(3, 2, 1, 0)
        w2a[RED, i] = b2g[gidx].transpose(2, 1, 0)
    w2a = np.ascontiguousarray(
        w2a.reshape(RED + 1, 2 * KK * 128)).astype(np.float16)

    fpad = np.pad(feature_map, ((0, 0), (0, 0), (3, 3), (3, 3))).astype(
        np.float16)
    # feat7[b, c, kw, r, x] = fpad[b, c, r, x + kw]
    feat7 = np.stack([fpad[:, :, :, kw:kw + W] for kw in range(KS)], axis=2)

    in_maps = []
    for core in range(N_CORES):
        b, q = divmod(core, 4)
        r0 = q * ROWS
        in_maps.append(dict(
            guide=np.ascontiguousarray(
                guide_map[b, :, r0:r0 + ROWS, :]).astype(np.float16),
            feat7=np.ascontiguousarray(feat7[b, :, :, r0:r0 + PROWS, :]),
            w1t=w1t, w2a=w2a,
            scl=scale.reshape(-1, 1), shf=shift.reshape(-1, 1),
        ))

    if _CACHED_NC is None:
        _CACHED_NC = _build_nc()
    nc = _CACHED_NC

    br = run_bass_kernel_spmd(
        nc, in_maps, list(range(N_CORES)), trace=TRACE,
    )
    LAST_RESULTS = br

    out = np.empty((2, C, H, W), np.float32)
    for core in range(N_CORES):
        b, q = divmod(core, 4)
        r0 = q * ROWS
        out[b, :, r0:r0 + ROWS, :] = br.results[core]["out"].astype(
            np.float32)
    return out


# revision 9
# speedup vs baseline: 1.4024x; 1.0143x over previous
"""CrossInvolution kernel for 8 Trainium2 NeuronCores.

Math (per batch b):
  t      = relu(bn(w1 @ guide))                       # [RED=64, H*W]
  weight = w2 @ t + b2                                # [G*K*K=784, H*W]
  out[c,p] = sum_k weight[g(c)*49+k, p] * x[c, p+dk] + x[c, p]

Sharding: 8 cores = 2 batches x 4 row-slices of 16 image rows each.
Each core computes its full pipeline on its slice (halo rows come in
via host-side padding); no cross-core communication.

Engine plan (v2):
  - PE: mm1, then one fused "broadcast" matmul per tap with
    lhsT = w2e[65,128] (w2 rearranged per channel; row 64 carries the
    b2 bias via a ones-row appended to t; center tap bias += 1 folds
    in the +x residual). PSUM directly holds per-channel tap weights.
  - Scalar (ACT): BN+relu on t, PSUM->SBUF fp16 weight copies.
  - DVE: the involution multiplies and adds only, all contiguous fp16
    SBUF ops (2x mode). x arrives as 7 kw-shifted contiguous copies
    DMA'd straight from HBM (host ships the padded fp16 feature map).
  - GPSIMD: unused (concurrent GPSIMD SBUF reads starve the DVE).
"""

import numpy as np

import concourse.bass as bass
import concourse.bacc as bacc
import concourse.mybir as mybir
import concourse.tile as tile
from concourse.bass_utils import run_bass_kernel_spmd

FP = mybir.dt.float32
HP = mybir.dt.float16
N_CORES = 8
C = 256
RED = 64
G = 16
GC = 16
KS = 7
KK = KS * KS  # 49
H = W = 64
ROWS = 16          # image rows per core
PIX = ROWS * W     # 1024 pixels per core
PROWS = ROWS + 6   # padded rows (halo 3 each side)
PW = W + 6         # padded width
XROW = PROWS * W   # 1408: one kw-shifted copy, rows contiguous at 64

TRACE = False
LAST_RESULTS = None

_CACHED_NC = None


def _build_nc():
    nc = bacc.Bacc(
        "TRN2",
        debug=False,
        target_bir_lowering=False,
        num_devices=N_CORES,
    )

    guide = nc.dram_tensor("guide", (C, ROWS, W), HP, kind="ExternalInput")
    # feat7[c, kw, r, x] = padded_feature[c, r, x + kw] (host-shifted, so
    # every load and every DVE multiply is contiguous at 64-wide rows)
    feat7 = nc.dram_tensor("feat7", (C, KS, PROWS, W), HP,
                           kind="ExternalInput")
    w1t = nc.dram_tensor("w1t", (C, RED), HP, kind="ExternalInput")
    # w2a[r, ((i*7+kw)*7+kh)*128 + c] = w2[(c//16+8i)*49 + kh*7+kw, r];
    # row RED carries b2 (+1 on center tap -> +x residual)
    w2a = nc.dram_tensor("w2a", (RED + 1, 2 * KK * 128), HP,
                         kind="ExternalInput")
    scl = nc.dram_tensor("scl", (RED, 1), FP, kind="ExternalInput")
    shf = nc.dram_tensor("shf", (RED, 1), FP, kind="ExternalInput")
    out = nc.dram_tensor("out", (C, ROWS, W), HP, kind="ExternalOutput")

    MUL = mybir.AluOpType.mult
    ADD = mybir.AluOpType.add
    # taps per PSUM chunk: {2,2,2,1} per kw-group (PSUM = 8 banks =
    # 4096 fp32; 2-tap chunk = 2048 fp32 = 4 banks, ping-ponged)
    CHUNKS = [(0, 2), (2, 2), (4, 2), (6, 1)]

    with tile.TileContext(nc) as tc:
        with (
            tc.tile_pool(name="consts", bufs=1) as consts,
            tc.tile_pool(name="big", bufs=1) as big,
            tc.tile_pool(name="work", bufs=3) as work,
            tc.tile_pool(name="psum", bufs=2, space="PSUM") as psum,
        ):
            # ---- ACT table preload: tiny dep-free op at t=0 ----
            warm = consts.tile([1, 1], FP)
            nc.vector.memset(warm, 0.0)
            nc.scalar.activation(warm, warm,
                                 mybir.ActivationFunctionType.Relu)

            # ---- loads ----
            scl_sb = consts.tile([RED, 1], FP)
            nc.sync.dma_start(scl_sb, scl.ap())
            shf_sb = consts.tile([RED, 1], FP)
            nc.sync.dma_start(shf_sb, shf.ap())
            w1t_sb = [consts.tile([128, RED], HP, tag=f"w1t{i}",
                                  name=f"w1t{i}") for i in range(2)]
            guide_sb = [big.tile([128, PIX], HP, tag=f"gd{i}", name=f"gd{i}")
                        for i in range(2)]
            for i in range(2):
                csl = slice(i * 128, (i + 1) * 128)
                nc.sync.dma_start(
                    guide_sb[i],
                    guide.ap()[csl].rearrange("p a b -> p (a b)"))
                nc.sync.dma_start(w1t_sb[i], w1t.ap()[i * 128:(i + 1) * 128])
            w2a_sb = consts.tile([RED + 1, 2 * KK * 128], HP)
            # per-(half, kw) chunks of w2a and xk stream in just-in-time
            # (emitted inside the kw loop, after the first groups' chunks)
            xk = [big.tile([128, KS * XROW], HP, tag=f"xk{i}", name=f"xk{i}")
                  for i in range(2)]

            def load_group(i, kw):
                csl = slice(i * 128, (i + 1) * 128)
                nc.sync.dma_start(
                    w2a_sb[:, (i * KS + kw) * KS * 128:
                           (i * KS + kw + 1) * KS * 128],
                    w2a.ap()[:, (i * KS + kw) * KS * 128:
                             (i * KS + kw + 1) * KS * 128])
                nc.sync.dma_start(
                    xk[i][:, kw * XROW:(kw + 1) * XROW],
                    feat7.ap()[csl, kw].rearrange("p a b -> p (a b)"))

            # first two groups' weight/feature chunks go on the queue now
            load_group(0, 0)
            load_group(0, 1)

            # ---- mm1 + BN/relu -> t (fp16), plus ones row for bias ----
            t_sb = big.tile([RED + 1, PIX], HP)
            nc.vector.memset(t_sb[RED:RED + 1, :], 1.0)
            for nh in range(2):
                nsl = slice(nh * 512, (nh + 1) * 512)
                t_ps = psum.tile([RED, 512], FP, tag="wb",
                                 padded_shape=[RED, 2048])
                for i in range(2):
                    nc.tensor.matmul(
                        t_ps,
                        w1t_sb[i],
                        guide_sb[i][:, nsl],
                        start=(i == 0),
                        stop=(i == 1),
                    )
                nc.scalar.activation(
                    t_sb[0:RED, nsl], t_ps,
                    mybir.ActivationFunctionType.Relu,
                    bias=shf_sb[:, :], scale=scl_sb[:, :],
                )

            # ---- involution ----
            for i in range(2):
                run = big.tile([128, PIX], HP, tag=f"run{i}", name=f"run{i}")
                osb = big.tile([128, PIX], HP, tag=f"osb{i}", name=f"osb{i}")
                for kw in range(KS):
                    # prefetch the group two steps ahead
                    nxt = i * KS + kw + 2
                    if nxt < 2 * KS:
                        load_group(nxt // KS, nxt % KS)
                    # broadcast matmuls + fp16 copy for the 7 kh-taps
                    wsb = work.tile([128, KS * PIX], HP, tag="wsb", bufs=2)
                    prod = work.tile([128, KS * PIX], HP, tag="prod", bufs=2)
                    head = (i == 0 and kw == 0)
                    for (k0, nk) in CHUNKS:
                        w_ps = psum.tile([128, nk * PIX], FP, tag="wb",
                                         padded_shape=[128, 2048])
                        for u in range(nk):
                            kh = k0 + u
                            lhs = w2a_sb[:, ((i * KS + kw) * KS + kh) * 128:
                                         ((i * KS + kw) * KS + kh + 1) * 128]
                            for nh in range(2):
                                nc.tensor.matmul(
                                    w_ps[:, u * PIX + nh * 512:
                                         u * PIX + nh * 512 + 512],
                                    lhs,
                                    t_sb[:, nh * 512:nh * 512 + 512],
                                    start=True,
                                    stop=True,
                                )
                        nc.scalar.activation(
                            wsb[:, k0 * PIX:(k0 + nk) * PIX], w_ps,
                            mybir.ActivationFunctionType.Copy,
                        )
                        if head:
                            # pipeline head: multiply per-chunk so the
                            # DVE starts before the whole group is copied
                            xs = bass.AP(
                                tensor=xk[i].tensor,
                                offset=xk[i].offset + kw * XROW + k0 * W,
                                ap=[xk[i].ap[0], [W, nk], [1, PIX]],
                            )
                            nc.vector.tensor_tensor(
                                prod[:, k0 * PIX:(k0 + nk) * PIX].rearrange(
                                    "p (a b) -> p a b", b=PIX),
                                xs,
                                wsb[:, k0 * PIX:(k0 + nk) * PIX].rearrange(
                                    "p (a b) -> p a b", b=PIX),
                                MUL)
                    if not head:
                        # multiply all 7 kh-taps: runs of 1024 contiguous
                        xs = bass.AP(
                            tensor=xk[i].tensor,
                            offset=xk[i].offset + kw * XROW,
                            ap=[xk[i].ap[0], [W, KS], [1, PIX]],
                        )
                        nc.vector.tensor_tensor(
                            prod.rearrange("p (a b) -> p a b", b=PIX),
                            xs,
                            wsb.rearrange("p (a b) -> p a b", b=PIX),
                            MUL)
                    # 7 -> 1 tree (contiguous fp16), accumulated into run
                    s2 = work.tile([128, 3 * PIX], HP, tag="s2", bufs=2)
                    nc.vector.tensor_tensor(
                        s2, prod[:, 0:3 * PIX], prod[:, 3 * PIX:6 * PIX], ADD)
                    ab = work.tile([128, 2 * PIX], HP, tag="ab", bufs=2)
                    nc.vector.tensor_tensor(
                        ab[:, 0:PIX], s2[:, 0:PIX], s2[:, PIX:2 * PIX], ADD)
                    nc.vector.tensor_tensor(
                        ab[:, PIX:2 * PIX], s2[:, 2 * PIX:3 * PIX],
                        prod[:, 6 * PIX:7 * PIX], ADD)
                    if kw == 0:
                        nc.vector.tensor_tensor(
                            run, ab[:, 0:PIX], ab[:, PIX:2 * PIX], ADD)
                    else:
                        g = work.tile([128, PIX], HP, tag="g", bufs=2)
                        nc.vector.tensor_tensor(
                            g, ab[:, 0:PIX], ab[:, PIX:2 * PIX], ADD)
                        dst = osb if kw == KS - 1 else run
                        nc.vector.tensor_tensor(dst, run, g, ADD)
                nc.sync.dma_start(
                    out.ap()[i * 128:(i + 1) * 128].rearrange(
                        "p a b -> p (a b)"),
                    osb)

    nc.compile()
    return nc


def kernel(**inputs):
    global _CACHED_NC, LAST_RESULTS
    feature_map = np.asarray(inputs["feature_map"], np.float32)
    guide_map = np.asarray(inputs["guide_map"], np.float32)
    w1 = np.asarray(inputs["w1"], np.float32)
    bn_gamma = np.asarray(inputs["bn_gamma"], np.float32)
    bn_beta = np.asarray(inputs["bn_beta"], np.float32)
    bn_mean = np.asarray(inputs["bn_mean"], np.float32)
    bn_var = np.asarray(inputs["bn_var"], np.float32)
    w2 = np.asarray(inputs["w2"], np.float32)
    b2 = np.asarray(inputs["b2"], np.float32)

    scale = bn_gamma / np.sqrt(bn_var + 1e-5)
    shift = bn_beta - bn_mean * scale
    w1t = np.ascontiguousarray(w1.T).astype(np.float16)    # [256, 64]

    # w2a[r, i, kw, kh, c] = w2[(c//16 + 8i)*49 + kh*7+kw, r]; row RED =
    # b2 bias (+1 on the center tap: folds the +x residual in)
    w2g = w2.reshape(G, KS, KS, RED)                        # [g, kh, kw, r]
    b2g = b2.reshape(G, KS, KS).copy()                      # [g, kh, kw]
    b2g[:, 3, 3] += 1.0
    w2a = np.zeros((RED + 1, 2, KS, KS, 128), np.float32)
    for i in range(2):
        gidx = np.arange(128) // GC + 8 * i                 # [c] -> g
        # [r, kw, kh, c]
        w2a[0:RED, i] = w2g[gidx].transpose(3, 2, 1, 0)
        w2a[RED, i] = b2g[gidx].transpose(2, 1, 0)
    w2a = np.ascontiguousarray(
        w2a.reshape(RED + 1, 2 * KK * 128)).astype(np.float16)

    fpad = np.pad(feature_map, ((0, 0), (0, 0), (3, 3), (3, 3))).astype(
        np.float16)
    # feat7[b, c, kw, r, x] = fpad[b, c, r, x + kw]
    feat7 = np.stack([fpad[:, :, :, kw:kw + W] for kw in range(KS)], axis=2)

    in_maps = []
    for core in range(N_CORES):
        b, q = divmod(core, 4)
        r0 = q * ROWS
        in_maps.append(dict(
            guide=np.ascontiguousarray(
                guide_map[b, :, r0:r0 + ROWS, :]).astype(np.float16),
            feat7=np.ascontiguousarray(feat7[b, :, :, r0:r0 + PROWS, :]),
            w1t=w1t, w2a=w2a,
            scl=scale.reshape(-1, 1), shf=shift.reshape(-1, 1),
        ))

    if _CACHED_NC is None:
        _CACHED_NC = _build_nc()
    nc = _CACHED_NC

    br = run_bass_kernel_spmd(
        nc, in_maps, list(range(N_CORES)), trace=TRACE,
    )
    LAST_RESULTS = br

    out = np.empty((2, C, H, W), np.float32)
    for core in range(N_CORES):
        b, q = divmod(core, 4)
        r0 = q * ROWS
        out[b, :, r0:r0 + ROWS, :] = br.results[core]["out"].astype(
            np.float32)
    return out
